# revision 56
# baseline (speedup 1.0000x reference)
"""Trainium2 Bass kernel for nn_IntergraphInteract (GNN message passing).

Math (reference):
    score_e = Xq[u_e] . W . Xt[v_e] + b         (per edge, E=500k)
    beta_e  = sigmoid(score_e); w_e = exp(score_e)
    norm[v] = eps + sum_{e->v} w_e
    Xt_new[v] = sum_{e->v} w_e*((1-beta)Xq[u_e] + beta*Xt[v]) / norm[v]
    Xt_new[v_cons] = Xq[u_cons]

Restructured:
    Z = Xt @ W^T  (so score_e = Xq[u_e] . Z[v_e]),  w = exp(score)
    a_e := w(1-beta) = w/(1+w) = sigmoid(score)     (single ACT op)
    With S'[e,j] = a_e * onehot(v_e==j), rhs columns [G | 1+w | w]:
      S'^T @ G      = A[j]    = sum a_e Xq[u_e]
      S'^T @ (1+w)  = norm[j] = sum w_e            (a*(1+w) == w)
      S'^T @ w      = s[j]    = sum w_e*beta_e     (a*w == w*beta)
    Xt_new[j] = (A[j] + s[j]*Xt[j]) / (norm[j] + eps)

Sharding: 20000 target nodes assigned to 160 buckets (8 cores x 20
frames, <=128 nodes each) by greedy balance on degree so every bucket
holds ~3125 edges => uniform 25 subtiles of 128 edges per frame. No
collectives.

Software pipeline (frame f at iteration i) so the DVE - the critical
engine at ~11.5us/frame - never stalls on ACT/PE:
  i=f-2: dma_gather Xq[u] rows bf16 (4 SWDGE queues); prefetch both
         host-built one-hot layouts ([j,e] for zsel lhsT, [e,j,t] for S')
  i=f-1: zsel_t = onehotT_t^T @ Z_f per subtile (PE, 2-subtile groups)
         + PSUM->SBUF bf16 copies (ACT)
  i=f  : score_t via ONE fused DVE affine_mul_reduce per subtile (the
         native TENSOR_TENSOR_REDUCE opcode has no uop on this silicon);
         w=exp(score) / cw cols / a=sigmoid(score)=1-1/(1+w) on ACT
         (Exp+Copy only: single ACT table)
  i=f+1: S' = onehotEJ * a (ONE frame-wide broadcast TT, bf16 2x)
  i=f+2: segment sums via PE bf16 matmuls (G-chain N=256 + cw-chain
         N=2 per subtile, same stationary back to back), interleaved
         with iteration f+3's zsel groups; combine + DMA out
Consensus overwrite + bucket un-permutation on host.
"""

import sys
import numpy as np
from ml_dtypes import bfloat16 as ml_bf16

for _p in ("/opt/trn_rl_repo",):
    if _p not in sys.path:
        sys.path.insert(0, _p)

NQ, NT, D, E = 10000, 20000, 256, 500000
NCORES = 8
NFRAMES = 20
P = 128
NT_PAD = NFRAMES * P           # 2560 slot rows per core
NBUCKETS = NCORES * NFRAMES    # 160
NQUEUES = 4
EPS = 1e-10
OOB = 999.0                    # v_rel padding value (matches no iota column)

_PROG_CACHE = {}


def _split_excess_waits(nc, maxw=1):
    """The installed walrus rejects instructions carrying more than `maxw`
    semaphore waits ("Too many sync wait commands"), but this bass/Tile
    version freely emits more. Hoist excess waits onto same-engine NOPs
    inserted immediately before the over-waiting instruction (same-engine
    program order makes this semantically equivalent)."""
    import bass_rust

    for bb in nc.main_func.blocks:
        insts = bb.instructions  # live list object
        i = 0
        while i < len(insts):
            inst = insts[i]
            si = inst.sync_info
            eng = inst.engine
            if (
                si is not None
                and si.on_wait
                and len(si.on_wait) > maxw
                and eng in nc.engines
            ):
                waits = list(si.on_wait)
                keep = waits[-maxw:]
                extra = waits[:-maxw]
                si.on_wait = keep
                pos = i
                for j in range(0, len(extra), maxw):
                    chunk = extra[j : j + maxw]
                    nop = nc.engines[eng].nop(nofuse=True, hint="wait_split").ins
                    cur_list = nc.cur_bb.bb.instructions
                    assert cur_list[-1] is nop
                    cur_list.pop()
                    nop.sync_info = bass_rust.SyncInfo(
                        on_wait=chunk, on_update=[]
                    )
                    insts.insert(pos, nop)
                    pos += 1
                    i += 1
            i += 1


def _install_swdge_queue_lane_patch():
    """Tile round-robins SWDGE completion sems DMASW0..7 ignoring queue_num,
    but the ucode locks each sem to one SWDGE queue. Partition the 8 lanes
    by queue: queue q uses lanes {q, q+4}."""
    import concourse.tile_sem_assignment as tsa

    if getattr(tsa.TileClockTick, "_queue_lane_patched", False):
        return
    orig = tsa.TileClockTick._assign_tick

    def patched(self, inst):
        if (
            inst.engine == tsa.mybir.EngineType.Pool
            and isinstance(inst, tsa.DMAInst)
            and not isinstance(inst, tsa.bass_isa.UserSyncedRemoteDMADescs)
        ):
            q = int(getattr(inst, "queue_num", 0) or 0)
            cnt = getattr(self, "_q_lane_cnt", None)
            if cnt is None:
                cnt = self._q_lane_cnt = {}
            k = cnt.get(q, 0)
            cnt[q] = k + 1
            self.next_sw_dma_idx = (q % 4) + 4 * (k % 2)
        return orig(self, inst)

    tsa.TileClockTick._assign_tick = patched
    tsa.TileClockTick._queue_lane_patched = True


def _build_program(t_list):
    """Build the SPMD bass program. t_list[f] = subtile count of frame f."""
    import concourse.bass as bass
    import concourse.mybir as mybir
    import concourse.tile as tile

    _install_swdge_queue_lane_patch()

    f32 = mybir.dt.float32
    bf16 = mybir.dt.bfloat16
    i16 = mybir.dt.int16
    Alu = mybir.AluOpType
    Act = mybir.ActivationFunctionType

    t_tot = sum(t_list)
    idxc = 8 * t_tot

    tf0 = t_list[0]
    assert all(t == tf0 for t in t_list), "uniform t_list expected"

    nc = bass.Bass(num_swdge_queues=NQUEUES)
    xqb = nc.declare_dram_parameter("xqb", [NQ, D], bf16, False)
    xtT = nc.declare_dram_parameter("xtT", [D, NT_PAD], bf16, False)
    xt = nc.declare_dram_parameter("xt", [NT_PAD, D], f32, False)
    wT = nc.declare_dram_parameter("wT", [D, D], bf16, False)
    bcol = nc.declare_dram_parameter("bcol", [P, 1], f32, False)
    uidx = nc.declare_dram_parameter("uidx", [P, idxc], i16, False)
    onehT = nc.declare_dram_parameter("onehT", [P, t_tot * P], bf16, False)
    onehEJ = nc.declare_dram_parameter("onehEJ", [P, t_tot * P], bf16, False)
    out = nc.declare_dram_parameter("out", [NT_PAD, D], f32, True)

    from concourse import library_config

    with tile.TileContext(nc) as tc:
        nc.gpsimd.load_library(library_config.mlp)
        with (
            tc.tile_pool(name="const", bufs=1) as const,
            tc.tile_pool(name="g", bufs=5) as gpool,
            tc.tile_pool(name="cw", bufs=3) as cwpool,
            tc.tile_pool(name="oh", bufs=3) as ohpool,
            tc.tile_pool(name="oe", bufs=5) as ohepool,
            tc.tile_pool(name="sp", bufs=3) as sppool,
            tc.tile_pool(name="sc", bufs=3) as spool,
            tc.tile_pool(name="zb", bufs=15) as zbfpool,
            tc.tile_pool(name="scr", bufs=2) as scrpool,
            tc.tile_pool(name="cb", bufs=2) as cbpool,
            tc.tile_pool(name="ps", bufs=2, space="PSUM") as ppool,
            tc.tile_pool(name="pw", bufs=2, space="PSUM") as pwpool,
            tc.tile_pool(name="zs", bufs=4, space="PSUM") as zspool,
        ):
            # ---- constants ----
            b_sb = const.tile([P, 1], f32)
            nc.sync.dma_start(out=b_sb[:], in_=bcol[:])
            # split the index upload so the frame-0/1 gathers (the ramp
            # critical path) only wait on the first 100KB
            uidx_sb = const.tile([P, idxc], i16)
            idx01 = 8 * (t_list[0] + t_list[1])
            nc.sync.dma_start(out=uidx_sb[:, 0:idx01], in_=uidx[:, 0:idx01])
            nc.sync.dma_start(out=uidx_sb[:, idx01:], in_=uidx[:, idx01:])
            z_bf = const.tile([P, NFRAMES, D], bf16)

            # ---- phase 1: Z = Xt_loc @ W^T -> resident SBUF bf16 ----
            # xtT preloaded whole (2 DMAs) so the PE chain never waits on
            # per-tile loads stuck behind the big one-hot prefetch DMAs
            xtT_sb = const.tile([P, 2, NT_PAD], bf16)
            nc.sync.dma_start(out=xtT_sb[:, 0, :], in_=xtT[0:P, :])
            nc.sync.dma_start(out=xtT_sb[:, 1, :], in_=xtT[P : 2 * P, :])
            wT_sb = const.tile([P, 2, D], bf16)
            nc.sync.dma_start(out=wT_sb[:, 0, :], in_=wT[0:P, :])
            nc.sync.dma_start(out=wT_sb[:, 1, :], in_=wT[P : 2 * P, :])

            def phase1_frame(m):
                # two Z-frames per PSUM tile + one paired ACT copy: fewer
                # sync points, so the 40-MM chain runs near back-to-back
                zp = zspool.tile([P, 2, D], f32, tag="zsel")
                for k in (0, 1):
                    nc.tensor.matmul(
                        out=zp[:, k, :],
                        lhsT=xtT_sb[:, 0, (m + k) * P : (m + k + 1) * P],
                        rhs=wT_sb[:, 0, :],
                        start=True, stop=False,
                    )
                    nc.tensor.matmul(
                        out=zp[:, k, :],
                        lhsT=xtT_sb[:, 1, (m + k) * P : (m + k + 1) * P],
                        rhs=wT_sb[:, 1, :],
                        start=False, stop=True,
                    )
                nc.scalar.activation(
                    z_bf[:, m : m + 2, :], zp[:, 0:2, :], Act.Copy
                )

            # ---- phase 2: edge processing, software-pipelined by frame ----
            _nreg_cache = {}

            def nreg(n):
                if n not in _nreg_cache:
                    _nreg_cache[n] = nc.gpsimd.to_reg(n)
                return _nreg_cache[n]

            col0s = [sum(t_list[:f]) for f in range(NFRAMES + 1)]
            qi = 0

            def emit_gathers(f):
                nonlocal qi
                tf = t_list[f]
                G = gpool.tile([P, tf, D], bf16, tag="G")
                if f == 0:
                    # frame 0 paces the ramp: small leading chunks so the
                    # first dots' gather semaphores clear early
                    bounds = [0, 2, 5, 9, 13, 17, 21, tf]
                    bounds = sorted(set(min(b, tf) for b in bounds))
                else:
                    # fine-grained chunks (2 per queue): descriptor-gen
                    # pipelines with the DMA and each subtile's gather
                    # semaphore clears sooner for the downstream dots
                    bounds = list(range(0, tf, 4)) + [tf]
                    bounds = sorted(set(bounds))
                for t0, t1 in zip(bounds, bounds[1:]):
                    nch = (t1 - t0) * P
                    icol0 = 8 * col0s[f]
                    nc.gpsimd.dma_gather(
                        G[:, t0:t1, :],
                        xqb[:, :],
                        uidx_sb[:, icol0 + 8 * t0 : icol0 + 8 * t1],
                        nch,
                        nreg(nch),
                        D,
                        queue_num=qi % NQUEUES,
                    )
                    qi += 1
                return G

            def emit_oh(f):
                # host-precomputed one-hot, S_T layout: oh[j, t*128+i] =
                # (slot of edge i of subtile t == j)
                tf = t_list[f]
                col0 = col0s[f]
                oh = ohpool.tile([P, tf * P], bf16, tag="oh")
                nc.sync.dma_start(
                    out=oh[:], in_=onehT[:, col0 * P : (col0 + tf) * P]
                )
                return oh

            def emit_ohej(f):
                # host-precomputed one-hot, S' layout: ohe[e, j, t] =
                # (slot of edge (e, t) == j); Sp = ohe * a broadcast
                tf = t_list[f]
                col0 = col0s[f]
                ohe = ohepool.tile([P, P, tf], bf16, tag="ohe")
                nc.sync.dma_start(
                    out=ohe[:], in_=onehEJ[:, col0 * P : (col0 + tf) * P]
                )
                return ohe

            def emit_combine(f, Ans, Acw):
                ns_sb = cbpool.tile([P, 2], f32, tag="ns")
                nc.scalar.activation(
                    ns_sb[:], Acw[:, 0:2], Act.Copy, bias=EPS
                )
                rec2 = cbpool.tile([P, 1], f32, tag="rec2")
                nc.vector.reciprocal(out=rec2[:], in_=ns_sb[:, 0:1])
                sxr = cbpool.tile([P, 1], f32, tag="sxr")
                nc.vector.tensor_tensor(
                    out=sxr[:], in0=ns_sb[:, 1:2], in1=rec2[:], op=Alu.mult
                )
                xt_f = cbpool.tile([P, D], f32, tag="xtf")
                nc.sync.dma_start(out=xt_f[:], in_=xt[f * P : (f + 1) * P, :])
                xtsc = cbpool.tile([P, D], f32, tag="xtsc")
                nc.scalar.activation(
                    xtsc[:], xt_f[:], Act.Copy, scale=sxr[:, 0:1]
                )
                outf = cbpool.tile([P, D], f32, tag="outf")
                nc.vector.scalar_tensor_tensor(
                    out=outf[:],
                    in0=Ans[:, 0:D],
                    scalar=rec2[:, 0:1],
                    in1=xtsc[:],
                    op0=Alu.mult,
                    op1=Alu.add,
                )
                nc.sync.dma_start(out=out[f * P : (f + 1) * P, :], in_=outf[:])

            # Software pipeline (frame f at iteration i):
            #   i = f-2: gather G(f), prefetch oh/ohe(f)
            #   i = f-1: zsel(f) on PE + zbf(f) PSUM->bf16 copies on ACT
            #   i = f  : amr dots -> score(f) on DVE (inputs all ready);
            #            w/cw/a on ACT
            #   i = f+1: Sp(f) = ohe*a on DVE
            #   i = f+2: segment matmuls (PE, interleaved with zsel(f+3))
            #            + combine(f) on DVE/ACT
            # This keeps every DVE op's inputs one full iteration old, so
            # the DVE (the critical engine) never stalls on ACT/PE.
            Gs, OHs, OHEs = {}, {}, {}
            zbfs = {}    # f -> list of (zbf, t0, t1)
            frames = {}  # f -> dict(G, cwt, Sp)
            seg_state = {}  # f -> [G, cwt, Sp, Ans, Acw, cursor]

            def open_seg(f):
                st = frames.pop(f)
                Ans = ppool.tile([P, D], f32, tag="Ans")
                Acw = pwpool.tile([P, 2], f32, tag="Acw")
                seg_state[f] = [st["G"], st["cwt"], st["Sp"], Ans, Acw, 0]

            def emit_seg_some(f, n):
                if f is None or f not in seg_state:
                    return
                G_, cw_, Sp_, Ans, Acw, cur = seg_state[f]
                qtf = t_list[f]
                hi = min(cur + n, qtf)
                for t in range(cur, hi):
                    nc.tensor.matmul(
                        out=Ans[:, 0:D], lhsT=Sp_[:, :, t], rhs=G_[:, t, :],
                        start=(t == 0), stop=(t == qtf - 1),
                    )
                    nc.tensor.matmul(
                        out=Acw[:, 0:2], lhsT=Sp_[:, :, t], rhs=cw_[:, t, :],
                        start=(t == 0), stop=(t == qtf - 1),
                    )
                seg_state[f][5] = hi

            def close_seg(f):
                if f is None or f not in seg_state:
                    return
                emit_seg_some(f, t_list[f])
                G_, cw_, Sp_, Ans, Acw, cur = seg_state.pop(f)
                emit_combine(f, Ans, Acw)

            def emit_zsel_zbf(f, seg_f, mid_cb=None):
                # PE: zsel groups of frame f, interleaved with segment
                # pairs of frame seg_f; ACT: PSUM->SBUF bf16 copies.
                # mid_cb fires after group 6 so the previous frame's w/c1
                # ACT ops land mid-copy-chain (their score input is ready
                # by then) instead of at the very end - this is what keeps
                # the DVE's end-of-iteration reciprocal from stalling.
                tf = t_list[f]
                oh = OHs.pop(f)
                groups = []
                for t0 in range(0, tf, 2):
                    t1 = min(t0 + 2, tf)
                    ng = t1 - t0
                    zp = zspool.tile([P, 2, D], f32, tag="zsel")
                    for t in range(t0, t1):
                        nc.tensor.matmul(
                            out=zp[:, t - t0, :],
                            lhsT=oh[:, t * P : (t + 1) * P],
                            rhs=z_bf[:, f, :],
                            start=True,
                            stop=True,
                        )
                    emit_seg_some(seg_f, 2)
                    if mid_cb is not None and t0 == 12:
                        mid_cb()
                    zbf = zbfpool.tile([P, 2, D], bf16, tag="zbf")
                    nc.scalar.activation(
                        zbf[:, 0:ng, :], zp[:, 0:ng, :], Act.Copy
                    )
                    groups.append((zbf, t0, t1))
                zbfs[f] = groups

            # prologue. One-hot prefetches are just-in-time (oh two
            # iterations before use, ohe two before its Sp build) so the
            # frame-0/1 gathers aren't starved of DMA bandwidth - the
            # first dot is gated by the gather-chunk semaphore.
            for m in range(0, NFRAMES, 2):
                phase1_frame(m)
            Gs[0] = emit_gathers(0)
            Gs[1] = emit_gathers(1)
            OHs[0] = emit_oh(0)
            OHs[1] = emit_oh(1)
            OHEs[0] = emit_ohej(0)
            emit_zsel_zbf(0, None)

            for f in range(NFRAMES):
                tf = t_list[f]
                if f + 2 < NFRAMES:
                    Gs[f + 2] = emit_gathers(f + 2)
                    OHs[f + 2] = emit_oh(f + 2)
                if f + 1 < NFRAMES:
                    OHEs[f + 1] = emit_ohej(f + 1)
                if f - 2 >= 0:
                    open_seg(f - 2)

                score = spool.tile([P, tf], f32, tag="score")
                w_sb = spool.tile([P, tf], f32, tag="w")
                c1 = spool.tile([P, tf], f32, tag="c1")

                def emit_wc1():
                    nc.scalar.activation(
                        w_sb[:], score[:], Act.Exp, bias=b_sb[:, 0:1]
                    )
                    nc.scalar.activation(c1[:], w_sb[:], Act.Copy, bias=1.0)

                # DVE: this frame's dots (emitted first so the mid_cb's
                # ACT reads of score see their writers)
                G = Gs.pop(f)
                scr = scrpool.tile([P, D], bf16, tag="scr")
                for zbf, t0, t1 in zbfs.pop(f):
                    for t in range(t0, t1):
                        # fused dot: out = in0*in1, accum_out = sum(out)
                        # (custom-DVE ucode op; the native
                        # TENSOR_TENSOR_REDUCE ISA opcode has no uop on
                        # this silicon and dies on HW)
                        nc.vector.affine_mul_reduce(
                            out=scr[:],
                            accum_out=score[:, t : t + 1],
                            in0=G[:, t, :],
                            in1=zbf[:, t - t0, :],
                            scale=1.0,
                            bias=0.0,
                        )

                # PE/ACT: next frame's zsel + copies, interleaved with the
                # deferred segment chain
                if f + 1 < NFRAMES:
                    emit_zsel_zbf(
                        f + 1, f - 2 if f - 2 >= 0 else None, mid_cb=emit_wc1
                    )
                elif f - 2 >= 0:
                    emit_seg_some(f - 2, t_list[f - 2])

                # w = exp(score + b); a = sigmoid(score+b) = 1 - 1/(1+w)
                # (computed via Exp+Copy+recip to stay on a single ACT
                # table); cw = [1+w | w] bf16. w/c1 were already emitted
                # mid-copy-chain via mid_cb except on the last iteration.
                if f + 1 >= NFRAMES:
                    emit_wc1()
                cwt = cwpool.tile([P, tf, 2], bf16, tag="cw")
                nc.scalar.activation(
                    cwt[:, :, 0:1], w_sb[:, :, None], Act.Copy, bias=1.0
                )
                nc.scalar.activation(cwt[:, :, 1:2], w_sb[:, :, None], Act.Copy)

                # Sp of the PREVIOUS frame (its a_bf is long done on ACT)
                if f - 1 >= 0:
                    fp = f - 1
                    Spp = sppool.tile([P, P, t_list[fp]], bf16, tag="Sp")
                    nc.vector.tensor_tensor(
                        out=Spp[:],
                        in0=OHEs.pop(fp)[:],
                        in1=frames[fp]["a_bf"][:, None, :].to_broadcast(
                            [P, P, t_list[fp]]
                        ),
                        op=Alu.mult,
                    )
                    frames[fp]["Sp"] = Spp

                if f - 2 >= 0:
                    close_seg(f - 2)

                # recip LAST in this iteration's DVE stream: by now its c1
                # input (ACT) is long done, so the DVE queue never blocks
                # on it; its a_bf consumer has a full iteration of slack
                rec = spool.tile([P, tf], f32, tag="rec")
                nc.vector.reciprocal(out=rec[:], in_=c1[:])
                a_bf = spool.tile([P, tf], bf16, tag="abf")
                nc.scalar.activation(
                    a_bf[:], rec[:], Act.Copy, bias=1.0, scale=-1.0
                )
                frames[f] = dict(G=G, cwt=cwt, a_bf=a_bf, Sp=None)

            # tail: Sp of the last frame, then the two remaining segments
            fp = NFRAMES - 1
            Spp = sppool.tile([P, P, t_list[fp]], bf16, tag="Sp")
            nc.vector.tensor_tensor(
                out=Spp[:],
                in0=OHEs.pop(fp)[:],
                in1=frames[fp]["a_bf"][:, None, :].to_broadcast(
                    [P, P, t_list[fp]]
                ),
                op=Alu.mult,
            )
            frames[fp]["Sp"] = Spp
            # interleave the two tail segment chains so the PE pipeline
            # stays full through the epilogue
            open_seg(NFRAMES - 2)
            open_seg(NFRAMES - 1)
            for _ in range(0, tf0, 2):
                emit_seg_some(NFRAMES - 2, 2)
                emit_seg_some(NFRAMES - 1, 2)
            close_seg(NFRAMES - 2)
            close_seg(NFRAMES - 1)

    _split_excess_waits(nc, maxw=1)
    # Raw Bass skips the Bacc pass that fills .instr bytes for extended-ISA
    # instructions (TTR, library load); without it walrus says "ISA wrong
    # length".
    mybir.codegen_inst_isa_subclasses(nc)
    return nc


def _wrap_idx(arr):
    """int16 gather-index layout: position i -> (partition i%16, col i//16),
    replicated to 128 partitions."""
    a = arr.astype(np.int16).reshape(-1, 16).T  # [16, L/16]
    return np.tile(a, (8, 1))


def _prep(u_idx, v_idx):
    """Assign v-nodes to 160 balanced buckets (8 cores x 20 frames), group
    edges by bucket, pad to t_list[f]*128. Returns per-core gather/one-hot
    arrays, the global t_list, and the slot permutation."""
    deg = np.bincount(v_idx, minlength=NT).astype(np.int64)
    order = np.argsort(-deg, kind="stable")

    import heapq

    heap = [(0, b, 0) for b in range(NBUCKETS)]  # (load, bucket, used_slots)
    heapq.heapify(heap)
    v2bucket = np.empty(NT, np.int64)
    v2slot = np.empty(NT, np.int64)
    for v in order:
        load, b, used = heapq.heappop(heap)
        v2bucket[v] = b
        v2slot[v] = used
        used += 1
        entry = (load + int(deg[v]), b, used)
        if used < P:
            heapq.heappush(heap, entry)
        else:
            heapq.heappush(heap, (1 << 60, b, used))  # bucket full
    # bucket b = c * NFRAMES + f; global slot row = c*NT_PAD + f*P + slot
    slot_of_v = (
        (v2bucket // NFRAMES) * NT_PAD + (v2bucket % NFRAMES) * P + v2slot
    )

    ecnt = np.bincount(v2bucket[v_idx], minlength=NBUCKETS)
    counts = ecnt.reshape(NCORES, NFRAMES)
    t_list = [max(1, int(-(-counts[:, f].max() // P))) for f in range(NFRAMES)]

    eb = v2bucket[v_idx]
    eorder = np.argsort(eb, kind="stable")
    us = u_idx[eorder].astype(np.int64)
    vslot = v2slot[v_idx][eorder]
    bnd = np.searchsorted(eb[eorder], np.arange(NBUCKETS + 1))

    jj = np.arange(P, dtype=np.int32)
    cores = []
    for c in range(NCORES):
        u_parts, vr_parts = [], []
        for f in range(NFRAMES):
            b = c * NFRAMES + f
            lo, hi = bnd[b], bnd[b + 1]
            n = hi - lo
            L = t_list[f] * P
            ua = np.zeros(L, np.int64)
            vra = np.full(L, OOB, np.float32)
            ua[:n] = us[lo:hi]
            vra[:n] = vslot[lo:hi].astype(np.float32)
            u_parts.append(_wrap_idx(ua))
            vr_parts.append(vra.reshape(t_list[f], P).T)
        vr_cat = np.concatenate(vr_parts, axis=1)
        # edge-major v_rel row (subtile-major); one-hot in S_T layout:
        # onehT[j, e] = (vrel[e] == j)
        vrel_row = np.ascontiguousarray(vr_cat.T).reshape(-1)  # [t_tot*128]
        onehT = (vrel_row[None, :] == jj[:, None]).astype(ml_bf16)
        # one-hot in S' layout, per frame [e, j, t] with (j, t) packed
        # j-major: onehEJ[e, col0*128 + j*tf + t] = (vrel[e, col0+t] == j)
        ej_parts = []
        col0 = 0
        for f in range(NFRAMES):
            tf = t_list[f]
            blk = vr_cat[:, col0 : col0 + tf]  # [128 e, tf]
            oh = (blk[:, None, :] == jj[None, :, None])  # [e, j, t]
            ej_parts.append(oh.reshape(P, P * tf))
            col0 += tf
        onehEJ = np.concatenate(ej_parts, axis=1).astype(ml_bf16)
        cores.append(
            dict(
                uidx=np.ascontiguousarray(np.concatenate(u_parts, axis=1)),
                onehT=np.ascontiguousarray(onehT),
                onehEJ=np.ascontiguousarray(onehEJ),
            )
        )
    return cores, t_list, slot_of_v


def make_in_maps(inputs):
    """Host preprocessing: full inputs -> per-core in_maps + t_list."""
    Xq = np.asarray(inputs["Xq"], np.float32)
    Xt = np.asarray(inputs["Xt"], np.float32)
    W = np.asarray(inputs["W"], np.float32)
    b = np.asarray(inputs["b"], np.float32)
    u_idx = np.asarray(inputs["u_idx"])
    v_idx = np.asarray(inputs["v_idx"])

    cores, t_list, slot_of_v = _prep(u_idx, v_idx)
    xq_bf = Xq.astype(ml_bf16)
    wTr = np.ascontiguousarray(W.T).astype(ml_bf16)
    bcol = np.full((P, 1), b[0], np.float32)

    # Xt rows scattered into slot order (full [NCORES*NT_PAD, D])
    xt_slots = np.zeros((NCORES * NT_PAD, D), np.float32)
    xt_slots[slot_of_v] = Xt

    in_maps = []
    for c in range(NCORES):
        xt_c = xt_slots[c * NT_PAD : (c + 1) * NT_PAD]
        in_maps.append(
            dict(
                xqb=xq_bf,
                xtT=np.ascontiguousarray(xt_c.T).astype(ml_bf16),
                xt=xt_c,
                wT=wTr,
                bcol=bcol,
                uidx=cores[c]["uidx"],
                onehT=cores[c]["onehT"],
                onehEJ=cores[c]["onehEJ"],
            )
        )
    return in_maps, t_list, slot_of_v


def kernel(**inputs):
    from concourse.bass_utils import run_bass_kernel_spmd

    in_maps, t_list, slot_of_v = make_in_maps(inputs)

    key = tuple(t_list)
    if key not in _PROG_CACHE:
        _PROG_CACHE[key] = _build_program(t_list)
    nc = _PROG_CACHE[key]

    res = run_bass_kernel_spmd(nc, in_maps, list(range(NCORES)))

    out_slots = np.concatenate(
        [np.asarray(res.results[c]["out"]) for c in range(NCORES)], axis=0
    )
    out = out_slots[slot_of_v]
    # consensus overwrite (host): Xt_new[v_cons] = Xq[u_cons]
    u_cons = np.asarray(inputs["u_cons"])
    v_cons = np.asarray(inputs["v_cons"])
    out[v_cons] = np.asarray(inputs["Xq"], np.float32)[u_cons]
    return out


# revision 58
# speedup vs baseline: 1.0277x; 1.0277x over previous
"""Trainium2 Bass kernel for nn_IntergraphInteract (GNN message passing).

Math (reference):
    score_e = Xq[u_e] . W . Xt[v_e] + b         (per edge, E=500k)
    beta_e  = sigmoid(score_e); w_e = exp(score_e)
    norm[v] = eps + sum_{e->v} w_e
    Xt_new[v] = sum_{e->v} w_e*((1-beta)Xq[u_e] + beta*Xt[v]) / norm[v]
    Xt_new[v_cons] = Xq[u_cons]

Restructured:
    Z = Xt @ W^T  (so score_e = Xq[u_e] . Z[v_e]),  w = exp(score)
    a_e := w(1-beta) = w/(1+w) = sigmoid(score)     (single ACT op)
    With S'[e,j] = a_e * onehot(v_e==j), rhs columns [G | 1+w | w]:
      S'^T @ G      = A[j]    = sum a_e Xq[u_e]
      S'^T @ (1+w)  = norm[j] = sum w_e            (a*(1+w) == w)
      S'^T @ w      = s[j]    = sum w_e*beta_e     (a*w == w*beta)
    Xt_new[j] = (A[j] + s[j]*Xt[j]) / (norm[j] + eps)

Sharding: 20000 target nodes assigned to 160 buckets (8 cores x 20
frames, <=128 nodes each) by greedy balance on degree so every bucket
holds ~3125 edges => uniform 25 subtiles of 128 edges per frame. No
collectives.

Software pipeline (frame f at iteration i) so the DVE - the critical
engine at ~11.5us/frame - never stalls on ACT/PE:
  i=f-2: dma_gather Xq[u] rows bf16 (4 SWDGE queues); prefetch both
         host-built one-hot layouts ([j,e] for zsel lhsT, [e,j,t] for S')
  i=f-1: zsel_t = onehotT_t^T @ Z_f per subtile (PE, 2-subtile groups)
         + PSUM->SBUF bf16 copies (ACT)
  i=f  : score_t via ONE fused DVE affine_mul_reduce per subtile (the
         native TENSOR_TENSOR_REDUCE opcode has no uop on this silicon);
         w=exp(score) / cw cols / a=sigmoid(score)=1-1/(1+w) on ACT
         (Exp+Copy only: single ACT table)
  i=f+1: S' = onehotEJ * a (ONE frame-wide broadcast TT, bf16 2x)
  i=f+2: segment sums via PE bf16 matmuls (G-chain N=256 + cw-chain
         N=2 per subtile, same stationary back to back), interleaved
         with iteration f+3's zsel groups; combine + DMA out
Consensus overwrite + bucket un-permutation on host.
"""

import sys
import numpy as np
from ml_dtypes import bfloat16 as ml_bf16

for _p in ("/opt/trn_rl_repo",):
    if _p not in sys.path:
        sys.path.insert(0, _p)

NQ, NT, D, E = 10000, 20000, 256, 500000
NCORES = 8
NFRAMES = 20
P = 128
NT_PAD = NFRAMES * P           # 2560 slot rows per core
NBUCKETS = NCORES * NFRAMES    # 160
NQUEUES = 4
EPS = 1e-10
OOB = 999.0                    # v_rel padding value (matches no iota column)

_PROG_CACHE = {}


def _split_excess_waits(nc, maxw=1):
    """The installed walrus rejects instructions carrying more than `maxw`
    semaphore waits ("Too many sync wait commands"), but this bass/Tile
    version freely emits more. Hoist excess waits onto same-engine NOPs
    inserted immediately before the over-waiting instruction (same-engine
    program order makes this semantically equivalent)."""
    import bass_rust

    for bb in nc.main_func.blocks:
        insts = bb.instructions  # live list object
        i = 0
        while i < len(insts):
            inst = insts[i]
            si = inst.sync_info
            eng = inst.engine
            if (
                si is not None
                and si.on_wait
                and len(si.on_wait) > maxw
                and eng in nc.engines
            ):
                waits = list(si.on_wait)
                keep = waits[-maxw:]
                extra = waits[:-maxw]
                si.on_wait = keep
                pos = i
                for j in range(0, len(extra), maxw):
                    chunk = extra[j : j + maxw]
                    nop = nc.engines[eng].nop(nofuse=True, hint="wait_split").ins
                    cur_list = nc.cur_bb.bb.instructions
                    assert cur_list[-1] is nop
                    cur_list.pop()
                    nop.sync_info = bass_rust.SyncInfo(
                        on_wait=chunk, on_update=[]
                    )
                    insts.insert(pos, nop)
                    pos += 1
                    i += 1
            i += 1


def _install_swdge_queue_lane_patch():
    """Tile round-robins SWDGE completion sems DMASW0..7 ignoring queue_num,
    but the ucode locks each sem to one SWDGE queue. Partition the 8 lanes
    by queue: queue q uses lanes {q, q+4}."""
    import concourse.tile_sem_assignment as tsa

    if getattr(tsa.TileClockTick, "_queue_lane_patched", False):
        return
    orig = tsa.TileClockTick._assign_tick

    def patched(self, inst):
        if (
            inst.engine == tsa.mybir.EngineType.Pool
            and isinstance(inst, tsa.DMAInst)
            and not isinstance(inst, tsa.bass_isa.UserSyncedRemoteDMADescs)
        ):
            q = int(getattr(inst, "queue_num", 0) or 0)
            cnt = getattr(self, "_q_lane_cnt", None)
            if cnt is None:
                cnt = self._q_lane_cnt = {}
            k = cnt.get(q, 0)
            cnt[q] = k + 1
            self.next_sw_dma_idx = (q % 4) + 4 * (k % 2)
        return orig(self, inst)

    tsa.TileClockTick._assign_tick = patched
    tsa.TileClockTick._queue_lane_patched = True


def _build_program(t_list):
    """Build the SPMD bass program. t_list[f] = subtile count of frame f."""
    import concourse.bass as bass
    import concourse.mybir as mybir
    import concourse.tile as tile

    _install_swdge_queue_lane_patch()

    f32 = mybir.dt.float32
    bf16 = mybir.dt.bfloat16
    i16 = mybir.dt.int16
    Alu = mybir.AluOpType
    Act = mybir.ActivationFunctionType

    t_tot = sum(t_list)
    idxc = 8 * t_tot

    tf0 = t_list[0]
    assert all(t == tf0 for t in t_list), "uniform t_list expected"

    nc = bass.Bass(num_swdge_queues=NQUEUES)
    xqb = nc.declare_dram_parameter("xqb", [NQ, D], bf16, False)
    xtT = nc.declare_dram_parameter("xtT", [D, NT_PAD], bf16, False)
    xt = nc.declare_dram_parameter("xt", [NT_PAD, D], f32, False)
    wT = nc.declare_dram_parameter("wT", [D, D], bf16, False)
    bcol = nc.declare_dram_parameter("bcol", [P, 1], f32, False)
    uidx = nc.declare_dram_parameter("uidx", [P, idxc], i16, False)
    onehT = nc.declare_dram_parameter("onehT", [P, t_tot * P], bf16, False)
    onehEJ = nc.declare_dram_parameter("onehEJ", [P, t_tot * P], bf16, False)
    out = nc.declare_dram_parameter("out", [NT_PAD, D], f32, True)

    from concourse import library_config

    with tile.TileContext(nc) as tc:
        nc.gpsimd.load_library(library_config.mlp)
        with (
            tc.tile_pool(name="const", bufs=1) as const,
            tc.tile_pool(name="g", bufs=5) as gpool,
            tc.tile_pool(name="cw", bufs=3) as cwpool,
            tc.tile_pool(name="oh", bufs=3) as ohpool,
            tc.tile_pool(name="oe", bufs=5) as ohepool,
            tc.tile_pool(name="sp", bufs=3) as sppool,
            tc.tile_pool(name="sc", bufs=3) as spool,
            tc.tile_pool(name="zb", bufs=15) as zbfpool,
            tc.tile_pool(name="scr", bufs=2) as scrpool,
            tc.tile_pool(name="cb", bufs=2) as cbpool,
            tc.tile_pool(name="ps", bufs=2, space="PSUM") as ppool,
            tc.tile_pool(name="pw", bufs=2, space="PSUM") as pwpool,
            tc.tile_pool(name="zs", bufs=4, space="PSUM") as zspool,
        ):
            # ---- constants ----
            b_sb = const.tile([P, 1], f32)
            nc.sync.dma_start(out=b_sb[:], in_=bcol[:])
            # split the index upload so the frame-0/1 gathers (the ramp
            # critical path) only wait on the first 100KB
            uidx_sb = const.tile([P, idxc], i16)
            idx01 = 8 * (t_list[0] + t_list[1])
            nc.sync.dma_start(out=uidx_sb[:, 0:idx01], in_=uidx[:, 0:idx01])
            nc.sync.dma_start(out=uidx_sb[:, idx01:], in_=uidx[:, idx01:])
            z_bf = const.tile([P, NFRAMES, D], bf16)

            # ---- phase 1: Z = Xt_loc @ W^T -> resident SBUF bf16 ----
            # xtT preloaded whole (2 DMAs) so the PE chain never waits on
            # per-tile loads stuck behind the big one-hot prefetch DMAs
            xtT_sb = const.tile([P, 2, NT_PAD], bf16)
            nc.sync.dma_start(out=xtT_sb[:, 0, :], in_=xtT[0:P, :])
            nc.sync.dma_start(out=xtT_sb[:, 1, :], in_=xtT[P : 2 * P, :])
            wT_sb = const.tile([P, 2, D], bf16)
            nc.sync.dma_start(out=wT_sb[:, 0, :], in_=wT[0:P, :])
            nc.sync.dma_start(out=wT_sb[:, 1, :], in_=wT[P : 2 * P, :])

            def phase1_frame(m):
                # two Z-frames per PSUM tile + one paired ACT copy: fewer
                # sync points, so the 40-MM chain runs near back-to-back
                zp = zspool.tile([P, 2, D], f32, tag="zsel")
                for k in (0, 1):
                    nc.tensor.matmul(
                        out=zp[:, k, :],
                        lhsT=xtT_sb[:, 0, (m + k) * P : (m + k + 1) * P],
                        rhs=wT_sb[:, 0, :],
                        start=True, stop=False,
                    )
                    nc.tensor.matmul(
                        out=zp[:, k, :],
                        lhsT=xtT_sb[:, 1, (m + k) * P : (m + k + 1) * P],
                        rhs=wT_sb[:, 1, :],
                        start=False, stop=True,
                    )
                nc.scalar.activation(
                    z_bf[:, m : m + 2, :], zp[:, 0:2, :], Act.Copy
                )

            # ---- phase 2: edge processing, software-pipelined by frame ----
            _nreg_cache = {}

            def nreg(n):
                if n not in _nreg_cache:
                    _nreg_cache[n] = nc.gpsimd.to_reg(n)
                return _nreg_cache[n]

            col0s = [sum(t_list[:f]) for f in range(NFRAMES + 1)]
            qi = 0

            def emit_gathers(f):
                nonlocal qi
                tf = t_list[f]
                G = gpool.tile([P, tf, D], bf16, tag="G")
                if f == 0:
                    # frame 0 paces the ramp: small leading chunks so the
                    # first dots' gather semaphores clear early
                    step = -(-tf // NQUEUES)
                    bounds = [0, 2, 5, step, 2 * step, 3 * step, tf]
                    bounds = sorted(set(min(b, tf) for b in bounds))
                else:
                    step = -(-tf // NQUEUES)
                    bounds = list(range(0, tf, step)) + [tf]
                for t0, t1 in zip(bounds, bounds[1:]):
                    nch = (t1 - t0) * P
                    icol0 = 8 * col0s[f]
                    nc.gpsimd.dma_gather(
                        G[:, t0:t1, :],
                        xqb[:, :],
                        uidx_sb[:, icol0 + 8 * t0 : icol0 + 8 * t1],
                        nch,
                        nreg(nch),
                        D,
                        queue_num=qi % NQUEUES,
                    )
                    qi += 1
                return G

            def emit_oh(f):
                # host-precomputed one-hot, S_T layout: oh[j, t*128+i] =
                # (slot of edge i of subtile t == j)
                tf = t_list[f]
                col0 = col0s[f]
                oh = ohpool.tile([P, tf * P], bf16, tag="oh")
                nc.sync.dma_start(
                    out=oh[:], in_=onehT[:, col0 * P : (col0 + tf) * P]
                )
                return oh

            def emit_ohej(f):
                # host-precomputed one-hot, S' layout: ohe[e, j, t] =
                # (slot of edge (e, t) == j); Sp = ohe * a broadcast
                tf = t_list[f]
                col0 = col0s[f]
                ohe = ohepool.tile([P, P, tf], bf16, tag="ohe")
                nc.sync.dma_start(
                    out=ohe[:], in_=onehEJ[:, col0 * P : (col0 + tf) * P]
                )
                return ohe

            def emit_combine(f, Ans, Acw):
                ns_sb = cbpool.tile([P, 2], f32, tag="ns")
                nc.scalar.activation(
                    ns_sb[:], Acw[:, 0:2], Act.Copy, bias=EPS
                )
                rec2 = cbpool.tile([P, 1], f32, tag="rec2")
                nc.vector.reciprocal(out=rec2[:], in_=ns_sb[:, 0:1])
                sxr = cbpool.tile([P, 1], f32, tag="sxr")
                nc.vector.tensor_tensor(
                    out=sxr[:], in0=ns_sb[:, 1:2], in1=rec2[:], op=Alu.mult
                )
                xt_f = cbpool.tile([P, D], f32, tag="xtf")
                nc.sync.dma_start(out=xt_f[:], in_=xt[f * P : (f + 1) * P, :])
                xtsc = cbpool.tile([P, D], f32, tag="xtsc")
                nc.scalar.activation(
                    xtsc[:], xt_f[:], Act.Copy, scale=sxr[:, 0:1]
                )
                outf = cbpool.tile([P, D], f32, tag="outf")
                nc.vector.scalar_tensor_tensor(
                    out=outf[:],
                    in0=Ans[:, 0:D],
                    scalar=rec2[:, 0:1],
                    in1=xtsc[:],
                    op0=Alu.mult,
                    op1=Alu.add,
                )
                nc.sync.dma_start(out=out[f * P : (f + 1) * P, :], in_=outf[:])

            # Software pipeline (frame f at iteration i):
            #   i = f-2: gather G(f), prefetch oh/ohe(f)
            #   i = f-1: zsel(f) on PE + zbf(f) PSUM->bf16 copies on ACT
            #   i = f  : amr dots -> score(f) on DVE (inputs all ready);
            #            w/cw/a on ACT
            #   i = f+1: Sp(f) = ohe*a on DVE
            #   i = f+2: segment matmuls (PE, interleaved with zsel(f+3))
            #            + combine(f) on DVE/ACT
            # This keeps every DVE op's inputs one full iteration old, so
            # the DVE (the critical engine) never stalls on ACT/PE.
            Gs, OHs, OHEs = {}, {}, {}
            zbfs = {}    # f -> list of (zbf, t0, t1)
            frames = {}  # f -> dict(G, cwt, Sp)
            seg_state = {}  # f -> [G, cwt, Sp, Ans, Acw, cursor]

            def open_seg(f):
                st = frames.pop(f)
                Ans = ppool.tile([P, D], f32, tag="Ans")
                Acw = pwpool.tile([P, 2], f32, tag="Acw")
                seg_state[f] = [st["G"], st["cwt"], st["Sp"], Ans, Acw, 0]

            def emit_seg_some(f, n):
                if f is None or f not in seg_state:
                    return
                G_, cw_, Sp_, Ans, Acw, cur = seg_state[f]
                qtf = t_list[f]
                hi = min(cur + n, qtf)
                for t in range(cur, hi):
                    nc.tensor.matmul(
                        out=Ans[:, 0:D], lhsT=Sp_[:, :, t], rhs=G_[:, t, :],
                        start=(t == 0), stop=(t == qtf - 1),
                    )
                    nc.tensor.matmul(
                        out=Acw[:, 0:2], lhsT=Sp_[:, :, t], rhs=cw_[:, t, :],
                        start=(t == 0), stop=(t == qtf - 1),
                    )
                seg_state[f][5] = hi

            def close_seg(f):
                if f is None or f not in seg_state:
                    return
                emit_seg_some(f, t_list[f])
                G_, cw_, Sp_, Ans, Acw, cur = seg_state.pop(f)
                emit_combine(f, Ans, Acw)

            def emit_zsel_zbf(f, seg_f, mid_cb=None):
                # PE: zsel groups of frame f, interleaved with segment
                # pairs of frame seg_f; ACT: PSUM->SBUF bf16 copies.
                # mid_cb fires after group 6 so the previous frame's w/c1
                # ACT ops land mid-copy-chain (their score input is ready
                # by then) instead of at the very end - this is what keeps
                # the DVE's end-of-iteration reciprocal from stalling.
                tf = t_list[f]
                oh = OHs.pop(f)
                groups = []
                for t0 in range(0, tf, 2):
                    t1 = min(t0 + 2, tf)
                    ng = t1 - t0
                    zp = zspool.tile([P, 2, D], f32, tag="zsel")
                    for t in range(t0, t1):
                        nc.tensor.matmul(
                            out=zp[:, t - t0, :],
                            lhsT=oh[:, t * P : (t + 1) * P],
                            rhs=z_bf[:, f, :],
                            start=True,
                            stop=True,
                        )
                    emit_seg_some(seg_f, 2)
                    if mid_cb is not None and t0 == 12:
                        mid_cb()
                    zbf = zbfpool.tile([P, 2, D], bf16, tag="zbf")
                    nc.scalar.activation(
                        zbf[:, 0:ng, :], zp[:, 0:ng, :], Act.Copy
                    )
                    groups.append((zbf, t0, t1))
                zbfs[f] = groups

            # prologue. One-hot prefetches are just-in-time (oh two
            # iterations before use, ohe two before its Sp build) so the
            # frame-0/1 gathers aren't starved of DMA bandwidth - the
            # first dot is gated by the gather-chunk semaphore.
            for m in range(0, NFRAMES, 2):
                phase1_frame(m)
            Gs[0] = emit_gathers(0)
            Gs[1] = emit_gathers(1)
            OHs[0] = emit_oh(0)
            OHs[1] = emit_oh(1)
            OHEs[0] = emit_ohej(0)
            emit_zsel_zbf(0, None)

            for f in range(NFRAMES):
                tf = t_list[f]
                if f + 2 < NFRAMES:
                    Gs[f + 2] = emit_gathers(f + 2)
                    OHs[f + 2] = emit_oh(f + 2)
                if f + 1 < NFRAMES:
                    OHEs[f + 1] = emit_ohej(f + 1)
                if f - 2 >= 0:
                    open_seg(f - 2)

                score = spool.tile([P, tf], f32, tag="score")
                w_sb = spool.tile([P, tf], f32, tag="w")
                c1 = spool.tile([P, tf], f32, tag="c1")

                def emit_wc1():
                    nc.scalar.activation(
                        w_sb[:], score[:], Act.Exp, bias=b_sb[:, 0:1]
                    )
                    nc.scalar.activation(c1[:], w_sb[:], Act.Copy, bias=1.0)

                # DVE: this frame's dots (emitted first so the mid_cb's
                # ACT reads of score see their writers)
                G = Gs.pop(f)
                scr = scrpool.tile([P, 1], bf16, tag="scr")
                for zbf, t0, t1 in zbfs.pop(f):
                    for t in range(t0, t1):
                        # fused dot: out = in0*in1, accum_out = sum(out)
                        # (custom-DVE ucode op; the native
                        # TENSOR_TENSOR_REDUCE ISA opcode has no uop on
                        # this silicon and dies on HW). The dead `out` is
                        # a stride-0 broadcast dummy: no full-tile write
                        # traffic / WAW footprint between consecutive dots
                        nc.vector.affine_mul_reduce(
                            out=scr.broadcast_to([P, D]),
                            accum_out=score[:, t : t + 1],
                            in0=G[:, t, :],
                            in1=zbf[:, t - t0, :],
                            scale=1.0,
                            bias=0.0,
                        )

                # PE/ACT: next frame's zsel + copies, interleaved with the
                # deferred segment chain
                if f + 1 < NFRAMES:
                    emit_zsel_zbf(
                        f + 1, f - 2 if f - 2 >= 0 else None, mid_cb=emit_wc1
                    )
                elif f - 2 >= 0:
                    emit_seg_some(f - 2, t_list[f - 2])

                # w = exp(score + b); a = sigmoid(score+b) = 1 - 1/(1+w)
                # (computed via Exp+Copy+recip to stay on a single ACT
                # table); cw = [1+w | w] bf16. w/c1 were already emitted
                # mid-copy-chain via mid_cb except on the last iteration.
                if f + 1 >= NFRAMES:
                    emit_wc1()
                cwt = cwpool.tile([P, tf, 2], bf16, tag="cw")
                nc.scalar.activation(
                    cwt[:, :, 0:1], w_sb[:, :, None], Act.Copy, bias=1.0
                )
                nc.scalar.activation(cwt[:, :, 1:2], w_sb[:, :, None], Act.Copy)

                # Sp of the PREVIOUS frame (its a_bf is long done on ACT)
                if f - 1 >= 0:
                    fp = f - 1
                    Spp = sppool.tile([P, P, t_list[fp]], bf16, tag="Sp")
                    nc.vector.tensor_tensor(
                        out=Spp[:],
                        in0=OHEs.pop(fp)[:],
                        in1=frames[fp]["a_bf"][:, None, :].to_broadcast(
                            [P, P, t_list[fp]]
                        ),
                        op=Alu.mult,
                    )
                    frames[fp]["Sp"] = Spp

                if f - 2 >= 0:
                    close_seg(f - 2)

                # recip LAST in this iteration's DVE stream: by now its c1
                # input (ACT) is long done, so the DVE queue never blocks
                # on it; its a_bf consumer has a full iteration of slack
                rec = spool.tile([P, tf], f32, tag="rec")
                nc.vector.reciprocal(out=rec[:], in_=c1[:])
                a_bf = spool.tile([P, tf], bf16, tag="abf")
                nc.scalar.activation(
                    a_bf[:], rec[:], Act.Copy, bias=1.0, scale=-1.0
                )
                frames[f] = dict(G=G, cwt=cwt, a_bf=a_bf, Sp=None)

            # tail: Sp of the last frame, then the two remaining segments
            fp = NFRAMES - 1
            Spp = sppool.tile([P, P, t_list[fp]], bf16, tag="Sp")
            nc.vector.tensor_tensor(
                out=Spp[:],
                in0=OHEs.pop(fp)[:],
                in1=frames[fp]["a_bf"][:, None, :].to_broadcast(
                    [P, P, t_list[fp]]
                ),
                op=Alu.mult,
            )
            frames[fp]["Sp"] = Spp
            # interleave the two tail segment chains so the PE pipeline
            # stays full through the epilogue
            open_seg(NFRAMES - 2)
            open_seg(NFRAMES - 1)
            for _ in range(0, tf0, 2):
                emit_seg_some(NFRAMES - 2, 2)
                emit_seg_some(NFRAMES - 1, 2)
            close_seg(NFRAMES - 2)
            close_seg(NFRAMES - 1)

    _split_excess_waits(nc, maxw=1)
    # Raw Bass skips the Bacc pass that fills .instr bytes for extended-ISA
    # instructions (TTR, library load); without it walrus says "ISA wrong
    # length".
    mybir.codegen_inst_isa_subclasses(nc)
    return nc


def _wrap_idx(arr):
    """int16 gather-index layout: position i -> (partition i%16, col i//16),
    replicated to 128 partitions."""
    a = arr.astype(np.int16).reshape(-1, 16).T  # [16, L/16]
    return np.tile(a, (8, 1))


def _prep(u_idx, v_idx):
    """Assign v-nodes to 160 balanced buckets (8 cores x 20 frames), group
    edges by bucket, pad to t_list[f]*128. Returns per-core gather/one-hot
    arrays, the global t_list, and the slot permutation."""
    deg = np.bincount(v_idx, minlength=NT).astype(np.int64)
    order = np.argsort(-deg, kind="stable")

    import heapq

    heap = [(0, b, 0) for b in range(NBUCKETS)]  # (load, bucket, used_slots)
    heapq.heapify(heap)
    v2bucket = np.empty(NT, np.int64)
    v2slot = np.empty(NT, np.int64)
    for v in order:
        load, b, used = heapq.heappop(heap)
        v2bucket[v] = b
        v2slot[v] = used
        used += 1
        entry = (load + int(deg[v]), b, used)
        if used < P:
            heapq.heappush(heap, entry)
        else:
            heapq.heappush(heap, (1 << 60, b, used))  # bucket full
    # bucket b = c * NFRAMES + f; global slot row = c*NT_PAD + f*P + slot
    slot_of_v = (
        (v2bucket // NFRAMES) * NT_PAD + (v2bucket % NFRAMES) * P + v2slot
    )

    ecnt = np.bincount(v2bucket[v_idx], minlength=NBUCKETS)
    counts = ecnt.reshape(NCORES, NFRAMES)
    t_list = [max(1, int(-(-counts[:, f].max() // P))) for f in range(NFRAMES)]

    eb = v2bucket[v_idx]
    eorder = np.argsort(eb, kind="stable")
    us = u_idx[eorder].astype(np.int64)
    vslot = v2slot[v_idx][eorder]
    bnd = np.searchsorted(eb[eorder], np.arange(NBUCKETS + 1))

    jj = np.arange(P, dtype=np.int32)
    cores = []
    for c in range(NCORES):
        u_parts, vr_parts = [], []
        for f in range(NFRAMES):
            b = c * NFRAMES + f
            lo, hi = bnd[b], bnd[b + 1]
            n = hi - lo
            L = t_list[f] * P
            ua = np.zeros(L, np.int64)
            vra = np.full(L, OOB, np.float32)
            ua[:n] = us[lo:hi]
            vra[:n] = vslot[lo:hi].astype(np.float32)
            u_parts.append(_wrap_idx(ua))
            vr_parts.append(vra.reshape(t_list[f], P).T)
        vr_cat = np.concatenate(vr_parts, axis=1)
        # edge-major v_rel row (subtile-major); one-hot in S_T layout:
        # onehT[j, e] = (vrel[e] == j)
        vrel_row = np.ascontiguousarray(vr_cat.T).reshape(-1)  # [t_tot*128]
        onehT = (vrel_row[None, :] == jj[:, None]).astype(ml_bf16)
        # one-hot in S' layout, per frame [e, j, t] with (j, t) packed
        # j-major: onehEJ[e, col0*128 + j*tf + t] = (vrel[e, col0+t] == j)
        ej_parts = []
        col0 = 0
        for f in range(NFRAMES):
            tf = t_list[f]
            blk = vr_cat[:, col0 : col0 + tf]  # [128 e, tf]
            oh = (blk[:, None, :] == jj[None, :, None])  # [e, j, t]
            ej_parts.append(oh.reshape(P, P * tf))
            col0 += tf
        onehEJ = np.concatenate(ej_parts, axis=1).astype(ml_bf16)
        cores.append(
            dict(
                uidx=np.ascontiguousarray(np.concatenate(u_parts, axis=1)),
                onehT=np.ascontiguousarray(onehT),
                onehEJ=np.ascontiguousarray(onehEJ),
            )
        )
    return cores, t_list, slot_of_v


def make_in_maps(inputs):
    """Host preprocessing: full inputs -> per-core in_maps + t_list."""
    Xq = np.asarray(inputs["Xq"], np.float32)
    Xt = np.asarray(inputs["Xt"], np.float32)
    W = np.asarray(inputs["W"], np.float32)
    b = np.asarray(inputs["b"], np.float32)
    u_idx = np.asarray(inputs["u_idx"])
    v_idx = np.asarray(inputs["v_idx"])

    cores, t_list, slot_of_v = _prep(u_idx, v_idx)
    xq_bf = Xq.astype(ml_bf16)
    wTr = np.ascontiguousarray(W.T).astype(ml_bf16)
    bcol = np.full((P, 1), b[0], np.float32)

    # Xt rows scattered into slot order (full [NCORES*NT_PAD, D])
    xt_slots = np.zeros((NCORES * NT_PAD, D), np.float32)
    xt_slots[slot_of_v] = Xt

    in_maps = []
    for c in range(NCORES):
        xt_c = xt_slots[c * NT_PAD : (c + 1) * NT_PAD]
        in_maps.append(
            dict(
                xqb=xq_bf,
                xtT=np.ascontiguousarray(xt_c.T).astype(ml_bf16),
                xt=xt_c,
                wT=wTr,
                bcol=bcol,
                uidx=cores[c]["uidx"],
                onehT=cores[c]["onehT"],
                onehEJ=cores[c]["onehEJ"],
            )
        )
    return in_maps, t_list, slot_of_v


def kernel(**inputs):
    from concourse.bass_utils import run_bass_kernel_spmd

    in_maps, t_list, slot_of_v = make_in_maps(inputs)

    key = tuple(t_list)
    if key not in _PROG_CACHE:
        _PROG_CACHE[key] = _build_program(t_list)
    nc = _PROG_CACHE[key]

    res = run_bass_kernel_spmd(nc, in_maps, list(range(NCORES)))

    out_slots = np.concatenate(
        [np.asarray(res.results[c]["out"]) for c in range(NCORES)], axis=0
    )
    out = out_slots[slot_of_v]
    # consensus overwrite (host): Xt_new[v_cons] = Xq[u_cons]
    u_cons = np.asarray(inputs["u_cons"])
    v_cons = np.asarray(inputs["v_cons"])
    out[v_cons] = np.asarray(inputs["Xq"], np.float32)[u_cons]
    return out
